# revision 1
# baseline (speedup 1.0000x reference)
"""Trainium2 Bass kernel for CLIP-style symmetric contrastive loss.

Problem: image_features [8192, 1024] f32, text_features [8192, 1024] f32.
  loss = 0.5 * (CE(logits, diag) + CE(logits.T, diag)),
  logits = cosine_similarity(img, txt) / 0.07.

Distribution: shard image rows across 8 NeuronCores. Each core m computes the
slab S_m = img_n[m] @ txt_n.T / T  ([1024, 8192]) against the full normalized
text matrix, reduces exp(S - C) along rows (local log-sum-exp) and along
columns (partial column sums), and a single [8194]-float AllReduce combines
the column sums plus the per-core scalar partials. Every core then finishes
the scalar loss locally.

The text matrix ships to the device pre-transposed ([D, N], bf16) so the
contraction dim lands on SBUF partitions with plain contiguous DMAs; its
normalization happens on-device in that layout (ACT squares + PE ones-matmul
partition reduction + per-chunk rsqrt scaling).

Math (C = 1/T upper-bounds every logit, so exp(S - C) <= 1 is stable):
  loss = C + (R + L - (2/T) * Draw) / (2N)
    R    = sum_i log sum_j exp(S_ij - C)
    L    = sum_j log sum_i exp(S_ij - C)
    Draw = sum_i cos(img_i, txt_i)
"""
import threading
from contextlib import ExitStack

import ml_dtypes
import numpy as np

import concourse.bacc as bacc
import concourse.bass as bass
import concourse.bass_isa as bass_isa
import concourse.mybir as mybir
import concourse.tile as tile
from concourse.bass_utils import run_bass_kernel_spmd

F32 = mybir.dt.float32
BF16 = mybir.dt.bfloat16
AF = mybir.ActivationFunctionType
ALU = mybir.AluOpType

N_CORES = 8
N = 8192
D = 1024
TEMPERATURE = 0.07


def build_nc(n=N, d=D, n_cores=N_CORES, no_collective=False, prep_only=False):
    """Build the SPMD Bass program (same program on every core)."""
    inv_t = float(1.0 / TEMPERATURE)
    cexp = float(1.0 / TEMPERATURE)          # stabilizer: max possible logit
    rows = n // n_cores                      # image rows per core
    P = 128
    rp = rows // P                           # row-tiles per core (8)
    kt = d // P                              # contraction tiles (8)
    CH = 512                                 # matmul free-dim chunk
    n_ch = n // CH                           # column chunks (16)
    cb_sz = min(4, n_ch)                     # chunks per psum block

    nc = bacc.Bacc("TRN2", target_bir_lowering=False, debug=False,
                   num_devices=n_cores)
    img = nc.dram_tensor("img", [rows, d], F32, kind="ExternalInput").ap()
    txt_t = nc.dram_tensor("txt_t", [d, n], BF16, kind="ExternalInput").ap()
    txt_own = nc.dram_tensor("txt_own", [rows, d], F32, kind="ExternalInput").ap()
    ones = nc.dram_tensor("ones", [P, P], F32, kind="ExternalInput").ap()
    ones_b = nc.dram_tensor("ones_b", [P, P], BF16, kind="ExternalInput").ap()
    ident = nc.dram_tensor("ident", [P, P], BF16, kind="ExternalInput").ap()
    out = nc.dram_tensor("out", [1, 1], F32, kind="ExternalOutput").ap()

    with tile.TileContext(nc) as tc:
        _body(tc, img, txt_t, txt_own, ones, ones_b, ident, out,
              n=n, d=d, rows=rows, P=P, rp=rp, kt=kt, CH=CH,
              n_ch=n_ch, cb_sz=cb_sz, inv_t=inv_t, cexp=cexp, n_cores=n_cores,
              no_collective=no_collective, prep_only=prep_only)
    nc.compile()
    return nc


def _body(tc, img, txt_t, txt_own, ones, ones_b, ident, out, *, n, d, rows, P,
          rp, kt, CH, n_ch, cb_sz, inv_t, cexp, n_cores, no_collective,
          prep_only):
    nc = tc.nc
    with ExitStack() as ctx:
        persist = ctx.enter_context(tc.tile_pool(name="persist", bufs=1))
        stage_f = ctx.enter_context(tc.tile_pool(name="stage_f", bufs=2))
        stage_b = ctx.enter_context(tc.tile_pool(name="stage_b", bufs=2))
        sqp = ctx.enter_context(tc.tile_pool(name="sqp", bufs=2))
        rbp = ctx.enter_context(tc.tile_pool(name="rbp", bufs=1))
        exp_p = ctx.enter_context(tc.tile_pool(name="exp_p", bufs=4))
        v1 = ctx.enter_context(tc.tile_pool(name="v1", bufs=6))
        csb_p = ctx.enter_context(tc.tile_pool(name="csb_p", bufs=1))
        rpp = ctx.enter_context(tc.tile_pool(name="rpp", bufs=2))
        psum = ctx.enter_context(tc.tile_pool(name="psum", bufs=4, space="PSUM"))
        ssq_ps = ctx.enter_context(tc.tile_pool(name="ssq_ps", bufs=2, space="PSUM"))
        tp_ps = ctx.enter_context(tc.tile_pool(name="tp_ps", bufs=2, space="PSUM"))
        dram = ctx.enter_context(tc.tile_pool(name="dram", bufs=1, space="DRAM"))

        txtT = persist.tile([P, kt, n], BF16, tag="txtT")       # [d-part, k, j]
        imgT = persist.tile([P, kt, rows], BF16, tag="imgT")    # [d-part, k, i]
        acc = persist.tile([P, n], F32, tag="acc")              # col partial sums
        vecs = persist.tile([P, 64], F32, tag="vecs")
        ones_sb = persist.tile([P, P], F32, tag="ones")
        ones_bsb = persist.tile([P, P], BF16, tag="ones_bsb")
        ident_sb = persist.tile([P, P], BF16, tag="ident")
        cs_sb = persist.tile([P, n // P], F32, tag="cs_sb")
        ln_cs = persist.tile([P, n // P], F32, tag="ln_cs")
        ebias = persist.tile([P, 1], F32, tag="ebias")

        cbuf = dram.tile([1, n + 64], F32, tag="cbuf")
        cbuf_out = dram.tile([1, n + 64], F32, tag="cbuf_out", addr_space="Shared")

        nc.sync.dma_start(ones_sb[:], ones[:])
        nc.sync.dma_start(ones_bsb[:], ones_b[:])
        nc.sync.dma_start(ident_sb[:], ident[:])
        nc.gpsimd.memset(ebias[:], float(-cexp))

        # vecs column map:
        RS = 0          # cols 0..rp-1   : per-row-tile rowsum(exp)
        DG = 8          # cols 8..8+rp-1 : per-row-tile diag cosine partials
        LNR = 16        # cols 16..: ln of rowsums
        SC = 56         # col 56: R partial, 57: Draw partial

        # --- Phase A: image prep (+ diag dot with own text rows) ------------
        for t in range(rp):
            img_raw = stage_f.tile([P, d], F32, tag="stage")
            nc.sync.dma_start(img_raw[:], img[t * P:(t + 1) * P, :])
            to_raw = stage_f.tile([P, d], F32, tag="stage")
            nc.sync.dma_start(to_raw[:], txt_own[t * P:(t + 1) * P, :])

            v = v1.tile([P, 8], F32, tag="v1")
            sq = stage_b.tile([P, d], BF16, tag="sq")
            nc.scalar.activation(sq[:], img_raw[:], AF.Square,
                                 accum_out=v[:, 0:1])
            nc.scalar.activation(v[:, 1:2], v[:, 0:1], AF.Sqrt)
            nc.vector.reciprocal(v[:, 2:3], v[:, 1:2])       # 1/||img_i||
            sq2 = stage_b.tile([P, d], BF16, tag="sq")
            nc.scalar.activation(sq2[:], to_raw[:], AF.Square,
                                 accum_out=v[:, 3:4])
            nc.scalar.activation(v[:, 4:5], v[:, 3:4], AF.Sqrt)
            nc.vector.reciprocal(v[:, 5:6], v[:, 4:5])       # 1/||txt_own_i||

            dot_scr = stage_b.tile([P, d], BF16, tag="sq")
            nc.vector.tensor_tensor(dot_scr[:], img_raw[:], to_raw[:], ALU.mult)
            nc.vector.tensor_reduce(v[:, 6:7], dot_scr[:],
                                    axis=mybir.AxisListType.X, op=ALU.add)
            nc.vector.tensor_tensor(v[:, 7:8], v[:, 2:3], v[:, 5:6], ALU.mult)
            nc.vector.tensor_tensor(vecs[:, DG + t:DG + t + 1], v[:, 6:7],
                                    v[:, 7:8], ALU.mult)     # diag cosine

            imgn_b = stage_b.tile([P, d], BF16, tag="nrm")
            nc.vector.tensor_scalar_mul(imgn_b[:], img_raw[:], v[:, 2:3])
            # transpose imgn_b [128 i, 1024 d] into imgT k-tiles via PE
            for k in range(kt):
                tp = tp_ps.tile([P, P], BF16, tag="tp")
                nc.tensor.transpose(tp[:], imgn_b[:, k * P:(k + 1) * P],
                                    ident_sb[:])
                nc.vector.tensor_copy(imgT[:, k, t * P:(t + 1) * P], tp[:])

        # --- Phase B: text load (pre-transposed bf16) + normalize in place --
        for k in range(kt):
            nc.sync.dma_start(txtT[:, k, :], txt_t[k * P:(k + 1) * P, :])
        for c in range(n_ch):
            sl = slice(c * CH, (c + 1) * CH)
            ssq = ssq_ps.tile([P, CH], F32, tag="ssq")
            for k in range(kt):
                sqc = sqp.tile([P, CH], BF16, tag="sqc")
                nc.scalar.activation(sqc[:], txtT[:, k, sl], AF.Square)
                nc.tensor.matmul(ssq[:], ones_bsb[:], sqc[:],
                                 start=(k == 0), stop=(k == kt - 1))
            nrm = rbp.tile([P, CH], BF16, tag="nrm_c")
            nc.scalar.activation(nrm[:], ssq[:], AF.Sqrt)
            rcp = rbp.tile([P, CH], F32, tag="rcp")
            nc.vector.reciprocal(rcp[:], nrm[:])
            rb = rbp.tile([P, CH], BF16, tag="rb")
            nc.vector.tensor_copy(rb[:], rcp[:])
            for k in range(kt):
                nc.vector.tensor_tensor(txtT[:, k, sl], txtT[:, k, sl],
                                        rb[:], ALU.mult)

        if prep_only:
            nc.vector.tensor_reduce(vecs[:, 30:31], txtT[:, 0, 0:CH],
                                    axis=mybir.AxisListType.X, op=ALU.add)
            nc.vector.tensor_reduce(vecs[:, 31:32], imgT[:, 0, 0:CH],
                                    axis=mybir.AxisListType.X, op=ALU.add)
            nc.sync.dma_start(out[0:1, 0:1], vecs[0:1, 30:31])
            return

        # --- Phase C: main matmul + exp + row/col reductions ----------------
        for p in range(rp):
            rparts = rpp.tile([P, n_ch], F32, tag="rp")
            for cb in range(n_ch // cb_sz):
                mms = []
                for _ci in range(cb_sz):
                    mm_t = psum.tile([P, CH], F32, tag="mm")
                    mms.append(mm_t)
                for k in range(kt):
                    for ci in range(cb_sz):
                        c = cb * cb_sz + ci
                        nc.tensor.matmul(
                            mms[ci][:],
                            imgT[:, k, p * P:(p + 1) * P],
                            txtT[:, k, c * CH:(c + 1) * CH],
                            start=(k == 0), stop=(k == kt - 1))
                for ci in range(cb_sz):
                    c = cb * cb_sz + ci
                    ex = exp_p.tile([P, CH], BF16, tag="exp")
                    nc.scalar.activation(ex[:], mms[ci][:], AF.Exp,
                                         bias=ebias[:, 0:1], scale=inv_t,
                                         accum_out=rparts[:, c:c + 1])
                    sl = slice(c * CH, (c + 1) * CH)
                    if p == 0:
                        nc.vector.tensor_copy(acc[:, sl], ex[:])
                    else:
                        nc.vector.tensor_tensor(acc[:, sl], acc[:, sl], ex[:],
                                                ALU.add)
            nc.vector.tensor_reduce(vecs[:, RS + p:RS + p + 1], rparts[:],
                                    axis=mybir.AxisListType.X, op=ALU.add)

        # --- Phase D: local scalars -----------------------------------------
        nc.scalar.activation(vecs[:, LNR:LNR + rp], vecs[:, RS:RS + rp], AF.Ln)
        nc.vector.tensor_reduce(vecs[:, 24:25], vecs[:, LNR:LNR + rp],
                                axis=mybir.AxisListType.X, op=ALU.add)
        nc.gpsimd.partition_all_reduce(vecs[:, SC:SC + 1], vecs[:, 24:25],
                                       channels=P, reduce_op=bass_isa.ReduceOp.add)
        nc.vector.tensor_reduce(vecs[:, 25:26], vecs[:, DG:DG + rp],
                                axis=mybir.AxisListType.X, op=ALU.add)
        nc.gpsimd.partition_all_reduce(vecs[:, SC + 1:SC + 2], vecs[:, 25:26],
                                       channels=P, reduce_op=bass_isa.ReduceOp.add)

        # column partial sums (reduce acc over partitions via ones-matmul)
        for c in range(n_ch):
            ps = psum.tile([P, CH], F32, tag="mm")
            nc.tensor.matmul(ps[:], ones_sb[:], acc[:, c * CH:(c + 1) * CH],
                             start=True, stop=True)
            csb = csb_p.tile([P, CH], F32, tag="csb")
            nc.vector.tensor_copy(csb[0:1, :], ps[0:1, :])
            nc.sync.dma_start(cbuf[0:1, c * CH:(c + 1) * CH], csb[0:1, :])
        nc.sync.dma_start(cbuf[0:1, n:n + 2], vecs[0:1, SC:SC + 2])

        # --- Phase E: AllReduce + finish -------------------------------------
        if no_collective:
            nc.sync.dma_start(cbuf_out[:], cbuf[:])
        else:
            nc.gpsimd.collective_compute(
                "AllReduce", ALU.add,
                replica_groups=[list(range(n_cores))],
                ins=[cbuf[:].opt()], outs=[cbuf_out[:].opt()])

        nc.sync.dma_start(
            cs_sb[:], cbuf_out[0:1, 0:n].rearrange("a (p x) -> (a p) x", p=P))
        nc.scalar.activation(ln_cs[:], cs_sb[:], AF.Ln)
        nc.vector.tensor_reduce(vecs[:, 26:27], ln_cs[:],
                                axis=mybir.AxisListType.X, op=ALU.add)
        nc.gpsimd.partition_all_reduce(vecs[:, 27:28], vecs[:, 26:27],
                                       channels=P, reduce_op=bass_isa.ReduceOp.add)
        rd = v1.tile([P, 8], F32, tag="v1")
        nc.sync.dma_start(rd[0:1, 0:2], cbuf_out[0:1, n:n + 2])

        # loss = cexp + (R + L - (2/T) * Draw) / (2N)
        fin = v1.tile([P, 8], F32, tag="v1")
        nc.vector.tensor_tensor(fin[0:1, 0:1], rd[0:1, 0:1],
                                vecs[0:1, 27:28], ALU.add)          # R + L
        nc.vector.tensor_scalar_mul(fin[0:1, 1:2], rd[0:1, 1:2],
                                    float(-2.0 * inv_t))            # -(2/T) Draw
        nc.vector.tensor_tensor(fin[0:1, 2:3], fin[0:1, 0:1],
                                fin[0:1, 1:2], ALU.add)
        nc.scalar.activation(fin[0:1, 3:4], fin[0:1, 2:3], AF.Copy,
                             bias=float(cexp), scale=float(1.0 / (2 * n)))
        nc.sync.dma_start(out[0:1, 0:1], fin[0:1, 3:4])


def make_in_maps(image_features, text_features, n=N, d=D, n_cores=N_CORES):
    image_features = np.asarray(image_features, dtype=np.float32)
    text_features = np.asarray(text_features, dtype=np.float32)
    rows = n // n_cores
    txt_t = np.ascontiguousarray(text_features.T).astype(ml_dtypes.bfloat16)
    ones = np.ones((128, 128), dtype=np.float32)
    ones_b = np.ones((128, 128), dtype=ml_dtypes.bfloat16)
    ident = np.eye(128, dtype=np.float32).astype(ml_dtypes.bfloat16)
    return [
        {
            "img": image_features[m * rows:(m + 1) * rows],
            "txt_t": txt_t,
            "txt_own": text_features[m * rows:(m + 1) * rows],
            "ones": ones,
            "ones_b": ones_b,
            "ident": ident,
        }
        for m in range(n_cores)
    ]


_CACHE = {}
_LOCK = threading.Lock()


def _get_nc():
    with _LOCK:
        if "nc" not in _CACHE:
            _CACHE["nc"] = build_nc()
        return _CACHE["nc"]


def kernel(image_features, text_features):
    image_features = np.asarray(image_features, dtype=np.float32)
    text_features = np.asarray(text_features, dtype=np.float32)
    assert image_features.shape == (N, D) and text_features.shape == (N, D)
    nc = _get_nc()
    in_maps = make_in_maps(image_features, text_features)
    res = run_bass_kernel_spmd(nc, in_maps, list(range(N_CORES)))
    val = np.float32(res.results[0]["out"][0, 0])
    return np.array(val, dtype=np.float32)



# revision 10
# speedup vs baseline: 1.5551x; 1.5551x over previous
"""Trainium2 Bass kernel for CLIP-style symmetric contrastive loss.

Problem: image_features [8192, 1024] f32, text_features [8192, 1024] f32.
  loss = 0.5 * (CE(logits, diag) + CE(logits.T, diag)),
  logits = cosine_similarity(img, txt) / 0.07.

Distribution: shard image rows across 8 NeuronCores. Each core m computes the
slab S_m = img_n[m] @ txt_n.T  ([1024, 8192] cosines) against the full
normalized text matrix in fp8 (DoubleRow perf-mode matmuls, 2x PE rate),
reduces exp(C*S - C + LOG_OFF) along rows (ACT accum) and columns (bf16 tree
adds + ones-matmul partition reduce), and one [N+2]-float AllReduce combines
column sums + per-core scalars. A dummy 16-float AllReduce is issued at kernel
start so the CC ring init (~50 us) overlaps compute instead of sitting on the
critical path.

The text matrix ships pre-transposed ([D, N]) and pre-quantized to fp8e4m3 by
the host; its normalization (column rsqrt scale) happens on-device per
512-column chunk, pipelined with the main matmul of the previous chunk. Image
slabs ship pre-transposed bf16; normalization + fp8 quantize on device.

Math (C = 1/T bounds every logit; LOG_OFF keeps exp outputs ~O(1) for bf16):
  loss = (C - LOG_OFF) + (R + L - 2C * Draw) / (2N)
    R    = sum_i log sum_j exp(C*s_ij - C + LOG_OFF)
    L    = same over columns
    Draw = sum_i cos(img_i, txt_i)
"""
import math
import threading
from contextlib import ExitStack

import ml_dtypes
import numpy as np

import concourse.bacc as bacc
import concourse.bass as bass
import concourse.bass_isa as bass_isa
import concourse.mybir as mybir
import concourse.tile as tile
from concourse.bass_utils import run_bass_kernel_spmd

F32 = mybir.dt.float32
BF16 = mybir.dt.bfloat16
FP8 = mybir.dt.float8e4
AF = mybir.ActivationFunctionType
ALU = mybir.AluOpType
DR = mybir.MatmulPerfMode.DoubleRow

N_CORES = 8
N = 8192
D = 1024
TEMPERATURE = 0.07


def build_nc(n=N, d=D, n_cores=N_CORES, no_collective=False):
    nc = bacc.Bacc("TRN2", target_bir_lowering=False, debug=False,
                   num_devices=n_cores)
    rows = n // n_cores
    imgT = nc.dram_tensor("imgT", [d, rows], BF16, kind="ExternalInput").ap()
    totT = nc.dram_tensor("totT", [d, rows], BF16, kind="ExternalInput").ap()
    txt8 = nc.dram_tensor("txt8", [d, n], FP8, kind="ExternalInput").ap()
    ones8 = nc.dram_tensor("ones8", [128, 256], FP8, kind="ExternalInput").ap()
    onesb = nc.dram_tensor("onesb", [128, 128], BF16, kind="ExternalInput").ap()
    out = nc.dram_tensor("out", [1, 1], F32, kind="ExternalOutput").ap()

    with tile.TileContext(nc) as tc:
        _body(tc, imgT, totT, txt8, ones8, onesb, out,
              n=n, d=d, rows=rows, n_cores=n_cores,
              no_collective=no_collective)
    nc.compile()
    return nc


def _body(tc, imgT, totT, txt8, ones8, onesb, out, *, n, d, rows, n_cores,
          no_collective):
    nc = tc.nc
    P = 128
    kt = d // P                      # 8 contraction tiles
    kp = kt // 2                     # 4 DoubleRow k-pairs
    CH = 512
    n_ch = n // CH                   # 16 column chunks
    rp = rows // P                   # 8 image row blocks
    rc_ch = rows // CH               # 2 phase-A column chunks
    inv_t = float(1.0 / TEMPERATURE)
    log_off = float(20.0 * math.log(2.0))
    ebias_v = float(log_off - inv_t)         # exp bias: -C + LOG_OFF
    W = n + 2                                # collective payload floats

    with ExitStack() as ctx:
        persist = ctx.enter_context(tc.tile_pool(name="persist", bufs=1))
        sq8p = ctx.enter_context(tc.tile_pool(name="sq8p", bufs=3))
        rsp = ctx.enter_context(tc.tile_pool(name="rsp", bufs=2))
        exsp = ctx.enter_context(tc.tile_pool(name="exsp", bufs=2))
        v1 = ctx.enter_context(tc.tile_pool(name="v1", bufs=1))
        psum_mm = ctx.enter_context(tc.tile_pool(name="psum_mm", bufs=4,
                                                 space="PSUM"))
        psum_n = ctx.enter_context(tc.tile_pool(name="psum_n", bufs=2,
                                                space="PSUM"))
        dram = ctx.enter_context(tc.tile_pool(name="dram", bufs=1,
                                              space="DRAM"))

        txt8s = persist.tile([P, kt, n], FP8, tag="txt8s")       # 8 MB
        img8 = persist.tile([P, kt, rows], FP8, tag="img8")      # 1 MB
        imgT_sb = persist.tile([P, kt, rows], BF16, tag="imgT")  # 2 MB
        totT_sb = persist.tile([P, kt, rows], BF16, tag="totT")  # 2 MB
        rs_i = persist.tile([P, rows], BF16, tag="rs_i")
        rs_to = persist.tile([P, rows], BF16, tag="rs_to")
        ddv = persist.tile([P, rows], F32, tag="ddv")
        rparts = persist.tile([P, rp * n_ch], F32, tag="rparts")
        vecs = persist.tile([P, 16], F32, tag="vecs")
        ones8_sb = persist.tile([P, 2, P], FP8, tag="ones8")
        onesb_sb = persist.tile([P, P], BF16, tag="onesb")
        ebias = persist.tile([P, 1], F32, tag="ebias")
        cs_sb = persist.tile([P, n // P], F32, tag="cs_sb")
        ln_cs = persist.tile([P, n // P], F32, tag="ln_cs")

        warm = dram.tile([1, 16], F32, tag="warm")
        warm_out = dram.tile([1, 16], F32, tag="warm_out", addr_space="Shared")
        cbuf = dram.tile([1, W], F32, tag="cbuf")
        cbuf_out = dram.tile([1, W], F32, tag="cbuf_out", addr_space="Shared")

        grp = [list(range(n_cores))]

        # --- warm-up dummy collective: absorbs CC init + core start skew ----
        wsb = v1.tile([1, 16], F32, tag="wsb")
        nc.gpsimd.memset(wsb[:], 0.0)
        nc.sync.dma_start(warm[:], wsb[:])
        if not no_collective:
            nc.gpsimd.collective_compute(
                "AllReduce", ALU.add, replica_groups=grp,
                ins=[warm[:].opt()], outs=[warm_out[:].opt()])

        # --- constants + input DMAs ----------------------------------------
        nc.sync.dma_start(ones8_sb[:, 0, :], ones8[:, 0:P])
        nc.sync.dma_start(ones8_sb[:, 1, :], ones8[:, P:2 * P])
        nc.sync.dma_start(onesb_sb[:], onesb[:])
        nc.gpsimd.memset(ebias[:], ebias_v)

        for k in range(kt):
            nc.sync.dma_start(imgT_sb[:, k, :], imgT[k * P:(k + 1) * P, :])
        GW = n // 4                      # text DMA column-group width
        for k in range(kt):              # first text group early
            nc.sync.dma_start(txt8s[:, k, 0:GW], txt8[k * P:(k + 1) * P, 0:GW])
        for k in range(kt):
            nc.sync.dma_start(totT_sb[:, k, :], totT[k * P:(k + 1) * P, :])
        for g in range(1, 4):
            sl = slice(g * GW, (g + 1) * GW)
            for k in range(kt):
                nc.sync.dma_start(txt8s[:, k, sl], txt8[k * P:(k + 1) * P, sl])

        def col_rsqrt(src_ap, dst_ap):
            """dst_ap [P, CH] (bf16, partition-replicated) =
            1/sqrt(colsum(src_ap^2)) for a [P, kt, CH] source slice."""
            sq = sq8p.tile([P, kt, CH], FP8, tag="sq8")
            nc.vector.tensor_tensor(sq[:], src_ap, src_ap, ALU.mult)
            ps = psum_n.tile([P, CH], F32, tag="nps")
            for t in range(kp):
                nc.tensor.matmul(ps[:], ones8_sb[:],
                                 sq[:, 2 * t:2 * t + 2, :],
                                 start=(t == 0), stop=(t == kp - 1),
                                 perf_mode=DR)
            r32 = rsp.tile([P, CH], F32, tag="r32")
            nc.vector.reciprocal_approx_fast(r32[:], ps[:])
            nc.scalar.activation(dst_ap, r32[:], AF.Sqrt)

        # --- Phase A: image (+ own-text) norms, quantize, diag dots --------
        for rc in range(rc_ch):
            sl = slice(rc * CH, (rc + 1) * CH)
            col_rsqrt(imgT_sb[:, :, sl], rs_i[:, sl])
            nc.vector.tensor_tensor(
                img8[:, :, sl], imgT_sb[:, :, sl],
                rs_i[:, sl].unsqueeze(1).broadcast_to((P, kt, CH)), ALU.mult)
            col_rsqrt(totT_sb[:, :, sl], rs_to[:, sl])
            prod = exsp.tile([P, kt, CH], BF16, tag="prod", bufs=1)
            nc.gpsimd.tensor_tensor(prod[:], imgT_sb[:, :, sl],
                                    totT_sb[:, :, sl], ALU.mult)
            dps = psum_n.tile([P, CH], F32, tag="cps", bufs=1)
            for k in range(kt):
                nc.tensor.matmul(dps[:], onesb_sb[:], prod[:, k, :],
                                 start=(k == 0), stop=(k == kt - 1))
            nc.vector.tensor_copy(ddv[:, sl], dps[:])

        # --- Phases B+C interleaved: per-chunk text prep + main matmul -----
        cs_pend = {}

        def emit_colsum(c):
            accc, csl = cs_pend.pop(c)
            cps = psum_n.tile([P, CH], F32, tag="cps", bufs=1)
            nc.tensor.matmul(cps[:], onesb_sb[:], accc[:], start=True,
                             stop=True)
            csr = rsp.tile([1, CH], F32, tag="csr")
            nc.vector.tensor_copy(csr[:], cps[0:1, :])
            nc.sync.dma_start(cbuf[0:1, csl], csr[:])

        for c in range(n_ch):
            csl = slice(c * CH, (c + 1) * CH)
            # text chunk normalize (in place, fp8)
            rst = rsp.tile([P, CH], BF16, tag="rst")
            col_rsqrt(txt8s[:, :, csl], rst[:])
            nc.vector.tensor_tensor(
                txt8s[:, :, csl], txt8s[:, :, csl],
                rst[:].unsqueeze(1).broadcast_to((P, kt, CH)), ALU.mult)

            exs = exsp.tile([P, rp, CH], BF16, tag="exs")
            for p in range(rp):
                if p == rp // 2 and (c - 1) in cs_pend:
                    emit_colsum(c - 1)   # mid-chunk: deps long resolved
                mm = psum_mm.tile([P, CH], F32, tag="mm")
                for t in range(kp):
                    nc.tensor.matmul(
                        mm[:],
                        img8[:, 2 * t:2 * t + 2, p * P:(p + 1) * P],
                        txt8s[:, 2 * t:2 * t + 2, csl],
                        start=(t == 0), stop=(t == kp - 1), perf_mode=DR)
                nc.scalar.activation(
                    exs[:, p, :], mm[:], AF.Exp, bias=ebias[:, 0:1],
                    scale=inv_t,
                    accum_out=rparts[:, p * n_ch + c:p * n_ch + c + 1])
            # bf16 tree reduce over the 8 row blocks -> column partials
            nc.gpsimd.tensor_tensor(exs[:, 0:4, :], exs[:, 0:4, :],
                                    exs[:, 4:8, :], ALU.add)
            nc.gpsimd.tensor_tensor(exs[:, 0:2, :], exs[:, 0:2, :],
                                    exs[:, 2:4, :], ALU.add)
            accc = rsp.tile([P, CH], BF16, tag="accc")
            nc.gpsimd.tensor_tensor(accc[:], exs[:, 0, :], exs[:, 1, :],
                                    ALU.add)
            cs_pend[c] = (accc, csl)
        emit_colsum(n_ch - 1)

        # --- local scalars --------------------------------------------------
        # R partial: rowsum over chunks, ln, sum over (q, p), partition-reduce
        rsum = v1.tile([P, rp], F32, tag="rsum")
        for p in range(rp):
            nc.vector.tensor_reduce(rsum[:, p:p + 1],
                                    rparts[:, p * n_ch:(p + 1) * n_ch],
                                    axis=mybir.AxisListType.X, op=ALU.add)
        lnr = v1.tile([P, rp], F32, tag="lnr")
        nc.scalar.activation(lnr[:], rsum[:], AF.Ln)
        nc.vector.tensor_reduce(vecs[:, 0:1], lnr[:],
                                axis=mybir.AxisListType.X, op=ALU.add)
        nc.gpsimd.partition_all_reduce(vecs[:, 1:2], vecs[:, 0:1], channels=P,
                                       reduce_op=bass_isa.ReduceOp.add)
        # Draw partial: diag cosines (replicated over partitions)
        nc.vector.tensor_tensor(ddv[:], ddv[:], rs_i[:], ALU.mult)
        nc.vector.tensor_tensor(ddv[:], ddv[:], rs_to[:], ALU.mult)
        nc.vector.tensor_reduce(vecs[:, 2:3], ddv[:],
                                axis=mybir.AxisListType.X, op=ALU.add)

        nc.sync.dma_start(cbuf[0:1, n:n + 1], vecs[0:1, 1:2])
        nc.sync.dma_start(cbuf[0:1, n + 1:n + 2], vecs[0:1, 2:3])

        # --- AllReduce + finish ---------------------------------------------
        if no_collective:
            nc.sync.dma_start(cbuf_out[:], cbuf[:])
        else:
            nc.gpsimd.collective_compute(
                "AllReduce", ALU.add, replica_groups=grp,
                ins=[cbuf[:].opt()], outs=[cbuf_out[:].opt()])

        nc.sync.dma_start(
            cs_sb[:], cbuf_out[0:1, 0:n].rearrange("a (p x) -> (a p) x", p=P))
        nc.scalar.activation(ln_cs[:], cs_sb[:], AF.Ln)
        nc.vector.tensor_reduce(vecs[:, 3:4], ln_cs[:],
                                axis=mybir.AxisListType.X, op=ALU.add)
        nc.gpsimd.partition_all_reduce(vecs[:, 4:5], vecs[:, 3:4], channels=P,
                                       reduce_op=bass_isa.ReduceOp.add)
        rd = v1.tile([P, 8], F32, tag="rd")
        nc.sync.dma_start(rd[0:1, 0:2], cbuf_out[0:1, n:n + 2])

        # loss = (C - LOG_OFF) + (R + L - 2C*Draw) / (2N)
        fin = v1.tile([P, 8], F32, tag="fin")
        nc.vector.tensor_tensor(fin[0:1, 0:1], rd[0:1, 0:1], vecs[0:1, 4:5],
                                ALU.add)                        # R + L
        nc.vector.tensor_scalar_mul(fin[0:1, 1:2], rd[0:1, 1:2],
                                    float(-2.0 * inv_t))        # -2C*Draw
        nc.vector.tensor_tensor(fin[0:1, 2:3], fin[0:1, 0:1], fin[0:1, 1:2],
                                ALU.add)
        nc.scalar.activation(fin[0:1, 3:4], fin[0:1, 2:3], AF.Copy,
                             bias=float(inv_t - log_off),
                             scale=float(1.0 / (2 * n)))
        nc.sync.dma_start(out[0:1, 0:1], fin[0:1, 3:4])


def make_in_maps(image_features, text_features, n=N, d=D, n_cores=N_CORES):
    image_features = np.asarray(image_features, dtype=np.float32)
    text_features = np.asarray(text_features, dtype=np.float32)
    rows = n // n_cores
    txt8 = np.ascontiguousarray(text_features.T).astype(ml_dtypes.float8_e4m3)
    ones8 = np.ones((128, 256), dtype=ml_dtypes.float8_e4m3)
    onesb = np.ones((128, 128), dtype=ml_dtypes.bfloat16)
    maps = []
    for m in range(n_cores):
        sl = slice(m * rows, (m + 1) * rows)
        maps.append({
            "imgT": np.ascontiguousarray(
                image_features[sl].T).astype(ml_dtypes.bfloat16),
            "totT": np.ascontiguousarray(
                text_features[sl].T).astype(ml_dtypes.bfloat16),
            "txt8": txt8,
            "ones8": ones8,
            "onesb": onesb,
        })
    return maps


_CACHE = {}
_LOCK = threading.Lock()


def _get_nc():
    with _LOCK:
        if "nc" not in _CACHE:
            _CACHE["nc"] = build_nc()
        return _CACHE["nc"]


def kernel(image_features, text_features):
    image_features = np.asarray(image_features, dtype=np.float32)
    text_features = np.asarray(text_features, dtype=np.float32)
    assert image_features.shape == (N, D) and text_features.shape == (N, D)
    nc = _get_nc()
    in_maps = make_in_maps(image_features, text_features)
    res = run_bass_kernel_spmd(nc, in_maps, list(range(N_CORES)))
    val = np.float32(res.results[0]["out"][0, 0])
    return np.array(val, dtype=np.float32)
